# revision 20
# baseline (speedup 1.0000x reference)
"""Trainium2 Bass kernel for nn_HeatmapBatch.

Reference computes: one-hot delta (value 10.0) per (batch, keypoint) at
integer coords (r, c) in a 256x256 image, then depthwise-convolves with a
shared 9x9 kernel.  Since each image holds exactly one delta, the output is
zeros everywhere except a 9x9 patch of 10*kernel2d[::-1,::-1] (XLA conv is
cross-correlation) centred at (r, c), clipped at the borders.

Device strategy (data-parallel over batch, 8 cores x 8 batches = 168
images per core):
  - Output per core is a column-padded [168*256, 264] tensor in FP16
    (rel tolerance is 2e-2; fp16 rounds at ~5e-4) so the scatter moves
    half the bytes; 4 pad columns each side so a patch row never wraps to
    the next row.  Host converts back to f32 and strips padding.
  - The runtime hands kernels pre-zeroed ExternalOutput buffers (documented
    contract in bass_utils/bass2jax), so the kernel only scatters patches.
  - EVERY patch is exactly one indirect-DMA descriptor: a contiguous
    2121-element span of the padded image covering 9 image rows (9 K-rows
    separated by 255 zeros; overwriting gap zeros with zeros is harmless).
    Patches clipped at the top/bottom border get a per-partition CUSTOM
    span baked on host: the span start row is clamped into the image
    (0 or 247) and the kernel rows are shifted inside the span, with zeros
    elsewhere — so clipped patches need no separate pass and every span
    stays in bounds.  The HW DGE pairs one offset per SBUF partition, so
    2 scatter calls cover 168 patches (126 + 42 partitions).
  - The span tables are built on host and DMA'd straight into SBUF, so no
    engine ever touches the data between the input DMA and the scatter:
    no vector ops, no memset, no SBUF read-after-DMA races.
  - Input DMAs issue in parallel: idx + the 42-partition table on the sync
    HW-DGE queue (the idx completion sem gates descriptor-gen), the
    126-partition table on the scalar HW-DGE queue.  The scatter does NOT
    wait on the table DMAs: table partition p lands ~3us before the first
    scatter packet that reads it (both stream in partition order, and the
    earliest scatter packet trails the idx sem + descriptor-gen by >2us).
  - A fallback variant zero-fills the output with big DMAs first, in case
    the pre-zeroed-output contract ever fails (detected by sampling).
Host does sharding/layout prep and the final gather/strip of the padding.
"""

import numpy as np


def _ensure_axon_hooks():
    """bass_utils imports antenv.axon_hooks when tracing is requested (e.g.
    BASS_TRACE=1 in the environment); some images lack that module.  Provide
    it best-effort so a tracing harness degrades gracefully instead of
    crashing.  Never raises."""
    try:
        import antenv.axon_hooks  # noqa: F401
        return
    except Exception:
        pass
    try:
        import sys
        import types

        import antenv

        mod = types.ModuleType("antenv.axon_hooks")
        _state = {"hook": None}
        mod.set_axon_ntff_profile_hook = lambda h: _state.__setitem__("hook", h)
        mod.get_axon_ntff_profile_hook = lambda: _state["hook"]
        sys.modules["antenv.axon_hooks"] = mod
        antenv.axon_hooks = mod
        try:
            from trn_agent_boot.trn_boot import _ntff_profile_via_ctypes

            mod.set_axon_ntff_profile_hook(
                _ntff_profile_via_ctypes("/opt/axon/libaxon_pjrt.so")
            )
        except Exception:
            pass
    except Exception:
        pass


_ensure_axon_hooks()

B, KP, H = 64, 21, 256
KS, PAD = 9, 4
NCORES = 8
BLOC = B // NCORES          # 8 batches per core
NPTS = BLOC * KP            # 168 images per core
QP = 126                    # partitions in scatter call 1 (call 2: 42)
Q2 = NPTS - QP              # 42
WPAD = H + 2 * PAD          # 264 padded columns
ROWS = NPTS * H             # 43008 image rows per core
SPAN = 8 * WPAD + KS        # 2121: contiguous span of one patch
SMAX = H - KS               # 247: last valid span start row

_NC_CACHE = {}


def _build_nc(zero_fill: bool):
    """Raw Bass: parallel input DMAs, two span-scatter calls (126 + 42
    partitions), manual semaphores.  No compute engines."""
    from concourse import bass, mybir

    nc = bass.Bass(target_bir_lowering=False)
    i32, f16 = mybir.dt.int32, mybir.dt.float16
    out = nc.dram_tensor("out", [ROWS, WPAD], f16, kind="ExternalOutput")
    idxs = nc.dram_tensor("idxs", [QP, 2], i32, kind="ExternalInput")
    tab = nc.dram_tensor("tab", [QP, SPAN], f16, kind="ExternalInput")
    tab2 = nc.dram_tensor("tab2", [Q2, SPAN], f16, kind="ExternalInput")

    nfill = 32  # 43008 rows / 1344 rows per fill DMA
    with (
        nc.Block() as block,
        nc.semaphore("s_ix") as s_ix,
        nc.semaphore("s_kv") as s_kv,
        nc.semaphore("s_d") as s_d,
        nc.semaphore("s_z") as s_z,
        nc.semaphore("s_f") as s_f,
        nc.sbuf_tensor("idx_t", [QP, 2], i32) as idx_t,
        nc.sbuf_tensor("tab_t", [QP, SPAN], f16) as tab_t,
        nc.sbuf_tensor("tab2_t", [Q2, SPAN], f16) as tab2_t,
        nc.sbuf_tensor("zt", [128, 2772], f16) as zt,
    ):

        @block.sync
        def _(sync):
            sync.dma_start(out=idx_t[:], in_=idxs[:]).then_inc(s_ix, 16)
            sync.dma_start(out=tab2_t[:], in_=tab2[:]).then_inc(s_kv, 16)
            if zero_fill:
                sync.wait_ge(s_z, 1)
                blk = 1344  # 1344*264*2B = 0.71 MB per fill DMA
                for i in range(nfill):
                    sync.dma_start(
                        out=out[i * blk:(i + 1) * blk, :], in_=zt[:, :]
                    ).then_inc(s_f, 16)

        @block.scalar
        def _(scalar):
            scalar.dma_start(out=tab_t[:], in_=tab[:]).then_inc(s_kv, 16)

        if zero_fill:

            @block.vector
            def _(vector):
                vector.memset(zt[:], 0.0).then_inc(s_z, 1)

        @block.gpsimd
        def _(g):
            g.wait_ge(s_ix, 16)
            if zero_fill:
                g.wait_ge(s_kv, 32)
                g.wait_ge(s_f, nfill * 16)
            g.indirect_dma_start(
                out=out[:],
                out_offset=bass.IndirectOffsetOnAxis(ap=idx_t[:, 0:1], axis=1),
                in_=tab_t[:],
                in_offset=None,
            ).then_inc(s_d, 16)
            g.indirect_dma_start(
                out=out[:],
                out_offset=bass.IndirectOffsetOnAxis(ap=idx_t[:Q2, 1:2],
                                                     axis=1),
                in_=tab2_t[:],
                in_offset=None,
            ).then_inc(s_d, 16)
            # no final wait: the Block-exit dge_drain blocks until the SWDGE
            # scatter queue has fully completed

    return nc


def _get_nc(zero_fill: bool):
    key = bool(zero_fill)
    if key not in _NC_CACHE:
        nc = _build_nc(zero_fill)
        if not nc.is_finalized():
            nc.finalize()
        _NC_CACHE[key] = nc
    return _NC_CACHE[key]


_T9 = np.arange(KS)


def _prep_core(xc, flip10):
    """Host-fused span starts + per-patch span tables for one core.

    The span for point p starts at image row s = clip(r-4, 0, 247); kernel
    row k of the patch sits at span row s+k-(r-4), so clipped patches get
    their rows shifted inside the span and zeros elsewhere."""
    r = xc[:, 0].astype(np.int64)
    c = xc[:, 1].astype(np.int64)
    s = np.clip(r - PAD, 0, SMAX)
    starts = (WPAD * (H * np.arange(NPTS) + s) + c).astype(np.int32)
    # kidx[p, t] = which kernel row lands on span row t (or out of range)
    kidx = s[:, None] + _T9[None, :] - r[:, None] + PAD      # [168, 9]
    valid = (kidx >= 0) & (kidx < KS)
    content = np.where(
        valid[:, :, None], flip10[np.clip(kidx, 0, KS - 1)], 0.0
    ).astype(np.float16)                                      # [168, 9, 9]
    span = np.zeros((NPTS, SPAN), np.float16)
    for t in range(KS):
        span[:, t * WPAD:t * WPAD + KS] = content[:, t]
    idxs = np.zeros((QP, 2), np.int32)
    idxs[:, 0] = starts[:QP]
    idxs[:Q2, 1] = starts[QP:]
    return {"idxs": idxs, "tab": span[:QP], "tab2": span[QP:]}


def _in_maps(x, kernel2d):
    x = np.asarray(x)
    flip10 = 10.0 * np.asarray(kernel2d, dtype=np.float32)[::-1, ::-1]
    xr = x.reshape(NCORES, NPTS, 2)
    maps = [_prep_core(xr[ci], flip10) for ci in range(NCORES)]
    return "span2", maps


def _assemble(results):
    full = np.empty((B, KP, H, H), np.float32)
    for ci, res in enumerate(results):
        o = res["out"][:ROWS].reshape(BLOC, KP, H, WPAD)
        full[ci * BLOC:(ci + 1) * BLOC] = o[:, :, :, PAD:PAD + H]
    return full


def _run(mode, zero_fill, maps, **kw):
    from concourse.bass_utils import run_bass_kernel_spmd

    nc = _get_nc(zero_fill)
    return run_bass_kernel_spmd(nc, maps, core_ids=list(range(NCORES)), **kw)


def _zero_contract_ok(x, results):
    """Sample must-be-zero cells to confirm outputs arrived pre-zeroed."""
    x = np.asarray(x).reshape(NCORES, NPTS, 2)
    rng = np.random.RandomState(0)
    for c in (0, NCORES - 1):
        o = results[c]["out"][:ROWS].reshape(NPTS, H, WPAD)
        for p in rng.choice(NPTS, 24, replace=False):
            r = x[c, p, 0]
            rows = np.arange(H)
            far = rows[(rows < r - PAD - 1) | (rows > r + PAD + 1)]
            # stay clear of the clamped span window as well
            far = far[(far < np.clip(r - PAD, 0, SMAX) - 1)
                      | (far > np.clip(r - PAD, 0, SMAX) + KS)]
            if len(far) < 8:
                continue
            sel = rng.choice(far, 8, replace=False)
            if np.any(o[p][sel] != 0.0):
                return False
    return True


def kernel(x, kernel2d):
    mode, maps = _in_maps(x, kernel2d)
    res = _run(mode, False, maps)
    if not _zero_contract_ok(x, res.results):
        # pre-zeroed-output contract failed; redo with explicit zero fill
        res = _run(mode, True, maps)
    return _assemble(res.results)


# revision 25
# speedup vs baseline: 1.1399x; 1.1399x over previous
"""Trainium2 Bass kernel for nn_HeatmapBatch.

Reference computes: one-hot delta (value 10.0) per (batch, keypoint) at
integer coords (r, c) in a 256x256 image, then depthwise-convolves with a
shared 9x9 kernel.  Since each image holds exactly one delta, the output is
zeros everywhere except a 9x9 patch of 10*kernel2d[::-1,::-1] (XLA conv is
cross-correlation) centred at (r, c), clipped at the borders.

Device strategy (data-parallel over batch, 8 cores x 8 batches = 168
images per core):
  - Output per core is a column-padded [168*256, 264] tensor in FP16
    (rel tolerance is 2e-2; fp16 rounds at ~5e-4) so the scatter moves
    half the bytes; 4 pad columns each side so a patch row never wraps to
    the next row.  Host converts back to f32 and strips padding.
  - The runtime hands kernels pre-zeroed ExternalOutput buffers (documented
    contract in bass_utils/bass2jax), so the kernel only scatters patches.
  - EVERY patch is exactly one indirect-DMA descriptor: a contiguous
    2121-element span of the padded image covering 9 image rows (9 K-rows
    separated by 255 zeros; overwriting gap zeros with zeros is harmless).
    Patches clipped at the top/bottom border get a per-partition CUSTOM
    span baked on host: the span start row is clamped into the image
    (0 or 247) and the kernel rows are shifted inside the span, with zeros
    elsewhere — so clipped patches need no separate pass and every span
    stays in bounds.  The HW DGE pairs one offset per SBUF partition, so
    2 scatter calls cover 168 patches (126 + 42 partitions).
  - The span tables are built on host and DMA'd straight into SBUF, so no
    engine ever touches the data between the input DMA and the scatter:
    no vector ops, no memset, no SBUF read-after-DMA races.
  - Input DMAs issue in parallel: idx + the 42-partition table on the sync
    HW-DGE queue (the idx completion sem gates descriptor-gen), the
    126-partition table on the scalar HW-DGE queue.  The scatter does NOT
    wait on the table DMAs: table partition p lands ~3us before the first
    scatter packet that reads it (both stream in partition order, and the
    earliest scatter packet trails the idx sem + descriptor-gen by >2us).
  - A fallback variant zero-fills the output with big DMAs first, in case
    the pre-zeroed-output contract ever fails (detected by sampling).
Host does sharding/layout prep and the final gather/strip of the padding.
"""

import numpy as np


def _ensure_axon_hooks():
    """bass_utils imports antenv.axon_hooks when tracing is requested (e.g.
    BASS_TRACE=1 in the environment); some images lack that module.  Provide
    it best-effort so a tracing harness degrades gracefully instead of
    crashing.  Never raises."""
    try:
        import antenv.axon_hooks  # noqa: F401
        return
    except Exception:
        pass
    try:
        import sys
        import types

        import antenv

        mod = types.ModuleType("antenv.axon_hooks")
        _state = {"hook": None}
        mod.set_axon_ntff_profile_hook = lambda h: _state.__setitem__("hook", h)
        mod.get_axon_ntff_profile_hook = lambda: _state["hook"]
        sys.modules["antenv.axon_hooks"] = mod
        antenv.axon_hooks = mod
        try:
            from trn_agent_boot.trn_boot import _ntff_profile_via_ctypes

            mod.set_axon_ntff_profile_hook(
                _ntff_profile_via_ctypes("/opt/axon/libaxon_pjrt.so")
            )
        except Exception:
            pass
    except Exception:
        pass


_ensure_axon_hooks()

B, KP, H = 64, 21, 256
KS, PAD = 9, 4
NCORES = 8
BLOC = B // NCORES          # 8 batches per core
NPTS = BLOC * KP            # 168 images per core
QP = 126                    # partitions in scatter call 1 (call 2: 42)
Q2 = NPTS - QP              # 42
WPAD = H + 2 * PAD          # 264 padded columns
ROWS = NPTS * H             # 43008 image rows per core
SPAN = 8 * WPAD + KS        # 2121: contiguous span of one patch
SMAX = H - KS               # 247: last valid span start row

_NC_CACHE = {}


def _build_nc(zero_fill: bool):
    """Raw Bass: parallel input DMAs, two span-scatter calls (126 + 42
    partitions), manual semaphores.  No compute engines."""
    from concourse import bass, mybir

    nc = bass.Bass(target_bir_lowering=False)
    i32, f16 = mybir.dt.int32, mybir.dt.float16
    out = nc.dram_tensor("out", [ROWS, WPAD], f16, kind="ExternalOutput")
    idxs = nc.dram_tensor("idxs", [QP, 2], i32, kind="ExternalInput")
    tab = nc.dram_tensor("tab", [QP, SPAN], f16, kind="ExternalInput")
    tab2 = nc.dram_tensor("tab2", [Q2, SPAN], f16, kind="ExternalInput")

    nfill = 32  # 43008 rows / 1344 rows per fill DMA
    with (
        nc.Block() as block,
        nc.semaphore("s_ix") as s_ix,
        nc.semaphore("s_kv") as s_kv,
        nc.semaphore("s_d") as s_d,
        nc.semaphore("s_z") as s_z,
        nc.semaphore("s_f") as s_f,
        nc.sbuf_tensor("idx_t", [QP, 2], i32) as idx_t,
        nc.sbuf_tensor("tab_t", [QP, SPAN], f16) as tab_t,
        nc.sbuf_tensor("tab2_t", [Q2, SPAN], f16) as tab2_t,
        nc.sbuf_tensor("zt", [128, 2772], f16) as zt,
    ):

        @block.sync
        def _(sync):
            # sync carries ONLY idx: anything queued behind it would delay
            # the completion-sem post that gates descriptor-gen
            sync.dma_start(out=idx_t[:], in_=idxs[:]).then_inc(s_ix, 16)
            if zero_fill:
                sync.wait_ge(s_z, 1)
                blk = 1344  # 1344*264*2B = 0.71 MB per fill DMA
                for i in range(nfill):
                    sync.dma_start(
                        out=out[i * blk:(i + 1) * blk, :], in_=zt[:, :]
                    ).then_inc(s_f, 16)

        @block.scalar
        def _(scalar):
            # tab alone, first on the scalar queue: partition p lands ~3.5us
            # before the call-1 scatter packet that reads it (both stream in
            # partition order; the earliest scatter packet trails the idx
            # sem + descriptor-gen)
            scalar.dma_start(out=tab_t[:], in_=tab[:]).then_inc(s_kv, 16)

        if zero_fill:

            @block.vector
            def _(vector):
                vector.memset(zt[:], 0.0).then_inc(s_z, 1)

        @block.gpsimd
        def _(g):
            # tab2 rides the SWDGE scatter queue itself: queue FIFO ensures
            # every tab2 packet is dispatched before any scatter packet, and
            # the call-2 packets additionally queue behind call 1's drain
            g.dma_start(out=tab2_t[:], in_=tab2[:]).then_inc(s_kv, 16)
            g.wait_ge(s_ix, 16)
            if zero_fill:
                g.wait_ge(s_kv, 32)
                g.wait_ge(s_f, nfill * 16)
            g.indirect_dma_start(
                out=out[:],
                out_offset=bass.IndirectOffsetOnAxis(ap=idx_t[:, 0:1], axis=1),
                in_=tab_t[:],
                in_offset=None,
            ).then_inc(s_d, 16)
            g.indirect_dma_start(
                out=out[:],
                out_offset=bass.IndirectOffsetOnAxis(ap=idx_t[:Q2, 1:2],
                                                     axis=1),
                in_=tab2_t[:],
                in_offset=None,
            ).then_inc(s_d, 16)
            # no final wait: the Block-exit dge_drain blocks until the SWDGE
            # scatter queue has fully completed

    return nc


def _get_nc(zero_fill: bool):
    key = bool(zero_fill)
    if key not in _NC_CACHE:
        nc = _build_nc(zero_fill)
        if not nc.is_finalized():
            nc.finalize()
        _NC_CACHE[key] = nc
    return _NC_CACHE[key]


_T9 = np.arange(KS)


def _prep_core(xc, flip10):
    """Host-fused span starts + per-patch span tables for one core.

    The span for point p starts at image row s = clip(r-4, 0, 247); kernel
    row k of the patch sits at span row s+k-(r-4), so clipped patches get
    their rows shifted inside the span and zeros elsewhere."""
    r = xc[:, 0].astype(np.int64)
    c = xc[:, 1].astype(np.int64)
    s = np.clip(r - PAD, 0, SMAX)
    starts = (WPAD * (H * np.arange(NPTS) + s) + c).astype(np.int32)
    # kidx[p, t] = which kernel row lands on span row t (or out of range)
    kidx = s[:, None] + _T9[None, :] - r[:, None] + PAD      # [168, 9]
    valid = (kidx >= 0) & (kidx < KS)
    content = np.where(
        valid[:, :, None], flip10[np.clip(kidx, 0, KS - 1)], 0.0
    ).astype(np.float16)                                      # [168, 9, 9]
    span = np.zeros((NPTS, SPAN), np.float16)
    for t in range(KS):
        span[:, t * WPAD:t * WPAD + KS] = content[:, t]
    idxs = np.zeros((QP, 2), np.int32)
    idxs[:, 0] = starts[:QP]
    idxs[:Q2, 1] = starts[QP:]
    return {"idxs": idxs, "tab": span[:QP], "tab2": span[QP:]}


def _in_maps(x, kernel2d):
    x = np.asarray(x)
    flip10 = 10.0 * np.asarray(kernel2d, dtype=np.float32)[::-1, ::-1]
    xr = x.reshape(NCORES, NPTS, 2)
    maps = [_prep_core(xr[ci], flip10) for ci in range(NCORES)]
    return "span2", maps


def _assemble(results):
    full = np.empty((B, KP, H, H), np.float32)
    for ci, res in enumerate(results):
        o = res["out"][:ROWS].reshape(BLOC, KP, H, WPAD)
        full[ci * BLOC:(ci + 1) * BLOC] = o[:, :, :, PAD:PAD + H]
    return full


def _run(mode, zero_fill, maps, **kw):
    from concourse.bass_utils import run_bass_kernel_spmd

    nc = _get_nc(zero_fill)
    return run_bass_kernel_spmd(nc, maps, core_ids=list(range(NCORES)), **kw)


def _zero_contract_ok(x, results):
    """Sample must-be-zero cells to confirm outputs arrived pre-zeroed."""
    x = np.asarray(x).reshape(NCORES, NPTS, 2)
    rng = np.random.RandomState(0)
    for c in (0, NCORES - 1):
        o = results[c]["out"][:ROWS].reshape(NPTS, H, WPAD)
        for p in rng.choice(NPTS, 24, replace=False):
            r = x[c, p, 0]
            rows = np.arange(H)
            far = rows[(rows < r - PAD - 1) | (rows > r + PAD + 1)]
            # stay clear of the clamped span window as well
            far = far[(far < np.clip(r - PAD, 0, SMAX) - 1)
                      | (far > np.clip(r - PAD, 0, SMAX) + KS)]
            if len(far) < 8:
                continue
            sel = rng.choice(far, 8, replace=False)
            if np.any(o[p][sel] != 0.0):
                return False
    return True


def kernel(x, kernel2d):
    mode, maps = _in_maps(x, kernel2d)
    res = _run(mode, False, maps)
    if not _zero_contract_ok(x, res.results):
        # pre-zeroed-output contract failed; redo with explicit zero fill
        res = _run(mode, True, maps)
    return _assemble(res.results)
